# revision 5
# baseline (speedup 1.0000x reference)
"""Bass/Trainium2 kernel for nn_GNN_v7 (gnn_message_passing).

Key structural fact of the reference model: the graph stage consumes only
``stacked[0]`` -- i.e. the final [1,1] output depends solely on row 0 of the
[262144, 28] input ``x`` (plus the weights).  The batch dimension of the
branch MLPs is dead code with respect to the output, so the kernel computes
row 0's pipeline only:

    7 branch MLPs (din -> 64 -> 128)  ->  node features [7, 128]
    2 ARMA conv layers on the fixed 7-node graph
    global max pool over nodes -> classifier MLP (128 -> 64 -> 1)

All weights plus the row-0 slices of x plus the dense normalized adjacency
(derived from the runtime ``edge_index`` input) are packed host-side into a
single [128, W] f32 blob laid out exactly as SBUF wants it, so the device
kernel is one blob DMA + ~35 tiny PE/ACT/DVE instructions.  Activations are
kept in a transposed [features, nodes] layout so ARMA biases become
per-partition scalars for the ACT engine and the node max-pool becomes a
free-axis reduce on the vector engine.

The same program runs replicated on all 8 cores (SPMD); core 0's output is
returned.
"""

import os
import sys

for _p in ("/opt/trn_rl_repo", "/root/.axon_site/_ro/trn_rl_repo"):
    if os.path.isdir(_p) and _p not in sys.path:
        sys.path.insert(0, _p)

import numpy as np

import concourse.mybir as mybir
from concourse import bacc
from concourse import tile
from concourse.bass_utils import run_bass_kernel_spmd

F32 = mybir.dt.float32
N_CORES = 8
N_NODES = 7

# ---- blob column layout (all f32, 128 partitions) ----
_C_WI1 = 0          # a1_Wi  [128,128]
_C_WR1 = 128        # a1_Wr  [128,128]
_C_WI2 = 256        # a2_Wi  [128,128]
_C_WR2 = 384        # a2_Wr  [128,128]
_C_CW1 = 512        # cls_W1 [128,64]
_C_W2 = 576         # branch W2s [64,128] x4 (lep, me, jet, hl)
_C_B2T = 1088       # per-branch output bias, broadcast to columns [128,7]
_C_A1B = 1095       # a1_b [128,1]
_C_A2B = 1096       # a2_b [128,1]
_C_AT = 1097        # A^T (normalized adjacency, transposed) [7,7]
_C_W1 = 1104        # branch W1s [din,64] x4
_C_B1 = 1360        # branch b1s [64,1] x4
_C_CB1 = 1364       # cls_b1 [64,1]
_C_CW2 = 1365       # cls_W2 [64,1]
_C_CB2 = 1366       # cls_b2 [1,1]
_C_X = 1367         # row-0 slices of x, one column per branch
_W = 1374

# branch table: (x col offset, din, W1/b1/W2 group index, x slice start)
_BRANCHES = [
    (0, 3, 0, 0),    # lepton
    (1, 2, 1, 3),    # missing energy
    (2, 4, 2, 5),    # jet1
    (3, 4, 2, 9),    # jet2
    (4, 4, 2, 13),   # jet3
    (5, 4, 2, 17),   # jet4
    (6, 7, 3, 21),   # high-level
]

_compiled = {}


def _build_nc():
    nc = bacc.Bacc("TRN2", debug=False, target_bir_lowering=False)
    blob_d = nc.dram_tensor("blob", [128, _W], F32, kind="ExternalInput")
    out_d = nc.dram_tensor("out", [1, 1], F32, kind="ExternalOutput")

    with tile.TileContext(nc) as tc:
        with (
            tc.tile_pool(name="sb", bufs=1) as sb,
            tc.tile_pool(name="ps", bufs=1, space="PSUM") as ps,
        ):
            blob = sb.tile([128, _W], F32)
            # split the blob load across several issuing engines so the
            # pieces land on different DMA queues and run concurrently
            bounds = [0, 344, 688, 1032, _W]
            engines = [nc.sync, nc.scalar, nc.gpsimd, nc.sync]
            for eng, c0, c1 in zip(engines, bounds[:-1], bounds[1:]):
                eng.dma_start(blob[:, c0:c1], blob_d[:, c0:c1])

            # ---- 7 branch MLPs -> nodeT accumulated per-column in PSUM ----
            node_ps = ps.tile([128, N_NODES], F32, tag="node")
            for i, (xc, din, g, _xs) in enumerate(_BRANCHES):
                h_ps = ps.tile([64, 1], F32, tag="hbr", bufs=2)
                nc.tensor.matmul(
                    h_ps,
                    blob[0:din, _C_W1 + 64 * g : _C_W1 + 64 * (g + 1)],
                    blob[0:din, _C_X + xc : _C_X + xc + 1],
                    start=True,
                    stop=True,
                )
                r_sb = sb.tile([64, 1], F32, tag=f"rbr{i}")
                nc.scalar.activation(
                    r_sb,
                    h_ps,
                    mybir.ActivationFunctionType.Relu,
                    bias=blob[0:64, _C_B1 + g : _C_B1 + g + 1],
                )
                nc.tensor.matmul(
                    node_ps[:, i : i + 1],
                    blob[0:64, _C_W2 + 128 * g : _C_W2 + 128 * (g + 1)],
                    r_sb,
                    start=True,
                    stop=True,
                )

            # nodeT = node_ps + b2 (per-branch bias, pre-broadcast host-side)
            xT = sb.tile([128, N_NODES], F32, tag="xT0")
            nc.vector.tensor_add(xT, node_ps, blob[:, _C_B2T : _C_B2T + N_NODES])

            # ---- 2 ARMA layers, transposed layout [128 feat, 7 nodes] ----
            for li, (cwi, cwr, cb) in enumerate(
                [(_C_WI1, _C_WR1, _C_A1B), (_C_WI2, _C_WR2, _C_A2B)]
            ):
                # h = x @ Wi in natural layout [7,128]
                h_ps = ps.tile([N_NODES, 128], F32, tag="ah")
                nc.tensor.matmul(
                    h_ps, xT, blob[:, cwi : cwi + 128], start=True, stop=True
                )
                h_sb = sb.tile([N_NODES, 128], F32, tag=f"ahs{li}")
                nc.vector.tensor_copy(h_sb, h_ps)
                # outT = (A @ h)^T + (x @ Wr)^T   [128, 7]
                o_ps = ps.tile([128, N_NODES], F32, tag="ao", bufs=1)
                nc.tensor.matmul(
                    o_ps,
                    h_sb,
                    blob[0:N_NODES, _C_AT : _C_AT + N_NODES],
                    start=True,
                    stop=False,
                    skip_group_check=True,
                )
                nc.tensor.matmul(
                    o_ps,
                    blob[:, cwr : cwr + 128],
                    xT,
                    start=False,
                    stop=True,
                    skip_group_check=True,
                )
                xT = sb.tile([128, N_NODES], F32, tag=f"xT{li + 1}")
                nc.scalar.activation(
                    xT,
                    o_ps,
                    mybir.ActivationFunctionType.Relu,
                    bias=blob[:, cb : cb + 1],
                )

            # ---- max over nodes + classifier ----
            pool_sb = sb.tile([128, 1], F32, tag="pool")
            nc.vector.tensor_reduce(
                pool_sb, xT, mybir.AxisListType.X, mybir.AluOpType.max
            )
            c1_ps = ps.tile([64, 1], F32, tag="c1")
            nc.tensor.matmul(
                c1_ps, blob[:, _C_CW1 : _C_CW1 + 64], pool_sb, start=True, stop=True
            )
            cr_sb = sb.tile([64, 1], F32, tag="cr")
            nc.scalar.activation(
                cr_sb,
                c1_ps,
                mybir.ActivationFunctionType.Relu,
                bias=blob[0:64, _C_CB1 : _C_CB1 + 1],
            )
            o_ps = ps.tile([1, 1], F32, tag="co")
            nc.tensor.matmul(
                o_ps, blob[0:64, _C_CW2 : _C_CW2 + 1], cr_sb, start=True, stop=True
            )
            out_sb = sb.tile([1, 1], F32, tag="osb")
            nc.vector.tensor_add(out_sb, o_ps, blob[0:1, _C_CB2 : _C_CB2 + 1])
            nc.sync.dma_start(out_d[:], out_sb)

    nc.compile()
    return nc


def _pack_blob(inputs: dict) -> np.ndarray:
    f = lambda k: np.asarray(inputs[k], dtype=np.float32)
    blob = np.zeros((128, _W), np.float32)
    blob[:, _C_WI1 : _C_WI1 + 128] = f("a1_Wi")
    blob[:, _C_WR1 : _C_WR1 + 128] = f("a1_Wr")
    blob[:, _C_WI2 : _C_WI2 + 128] = f("a2_Wi")
    blob[:, _C_WR2 : _C_WR2 + 128] = f("a2_Wr")
    blob[:, _C_CW1 : _C_CW1 + 64] = f("cls_W1")
    names = ["lep", "me", "jet", "hl"]
    for g, n in enumerate(names):
        blob[0:64, _C_W2 + 128 * g : _C_W2 + 128 * (g + 1)] = f(f"{n}_W2")
        w1 = f(f"{n}_W1")
        blob[0 : w1.shape[0], _C_W1 + 64 * g : _C_W1 + 64 * (g + 1)] = w1
        blob[0:64, _C_B1 + g] = f(f"{n}_b1")
    for i, (_xc, _din, g, _xs) in enumerate(_BRANCHES):
        blob[:, _C_B2T + i] = f(f"{names[g]}_b2")
    blob[:, _C_A1B] = f("a1_b")
    blob[:, _C_A2B] = f("a2_b")

    # dense normalized adjacency from the runtime edge_index
    ei = np.asarray(inputs["edge_index"])
    src, dst = ei[0].astype(np.int64), ei[1].astype(np.int64)
    deg = np.zeros(N_NODES, np.float32)
    np.add.at(deg, dst, np.float32(1.0))
    dinv = np.where(deg > 0, deg ** -0.5, 0.0).astype(np.float32)
    norm = (dinv[src] * dinv[dst]).astype(np.float32)
    A = np.zeros((N_NODES, N_NODES), np.float32)
    np.add.at(A, (dst, src), norm)
    blob[0:N_NODES, _C_AT : _C_AT + N_NODES] = A.T

    blob[0:64, _C_CB1] = f("cls_b1")
    blob[0:64, _C_CW2] = f("cls_W2")[:, 0]
    blob[0, _C_CB2] = f("cls_b2")[0]

    x0 = f("x")[0]
    for xc, din, _g, xs in _BRANCHES:
        blob[0:din, _C_X + xc] = x0[xs : xs + din]
    return blob


def _get_nc():
    if "nc" not in _compiled:
        _compiled["nc"] = _build_nc()
    return _compiled["nc"]


def run(inputs: dict, **spmd_kwargs):
    """Run on hardware; returns (out [1,1] np.float32, BassKernelResults)."""
    nc = _get_nc()
    blob = _pack_blob(inputs)
    in_maps = [{"blob": blob} for _ in range(N_CORES)]
    res = run_bass_kernel_spmd(nc, in_maps, list(range(N_CORES)), **spmd_kwargs)
    out = np.asarray(res.results[0]["out"], dtype=np.float32).reshape(1, 1)
    return out, res


def kernel(**inputs) -> np.ndarray:
    out, _ = run(inputs)
    return out
